# revision 16
# baseline (speedup 1.0000x reference)
"""GAT (2-layer graph attention) Trainium2 kernel, v1 redesign.

Sharding (SPMD, 8 cores): batch b = core//2; within a core pair the 4
attention heads are split 2+2 (pure-data differences via packed per-core
weights). Pair-local collectives stitch halves between layers: AllGather
for the layer-0 head-concat, AllReduce(add) for the layer-1 head-mean.

Score pipeline (both local heads, both layers) uses the exp-factorized
form  P = m . [H a1 b1 + (1-H) a2 b2],  H = 1{ed_i + es_j >= 0},
a = exp(ed)-type per-target factors, b = exp(es)-type per-source factors:
  - H tile:   DVE tensor_scalar is_ge (bf16, 4x mode), per (h, jb, ih)
  - G1 = H*m: DVE tensor_tensor (bf16, 2x), some tiles on GpSimd via a
              fused scalar_tensor_tensor (is_ge then mult)
  - aggregation: three PE streams per layer in [j_part, i_free] layout,
    each i-chunked to 1024 so 3 accumulators fit PSUM:
      m-stream: moving gT, stationary [x0*e^{.2es}|e^{.2es}] both heads
      G1-stream per head: moving G1, stationary [x0*e^{es}|e^{es}|
                                                 x0*e^{.2es}|e^{.2es}]
    The m-stream is emitted first per i-half: it has no DVE dependency,
    so the PE starts immediately and HAM stays warm.
  - e_dst broadcast rows (edb) come from a PE rank-1 matmul against a
    host-tiled Wadb (all 128 columns equal W@ad_h) -- no gpsimd
    partition_broadcast.
Tails transpose [99, 125] blocks back to node-major and combine with
per-partition ACT scales a1/a2; softmax Z rides along as the 33rd row
of each stationary (ones column scaled by e^{es}-factors).
"""
import numpy as np
import ml_dtypes
from contextlib import ExitStack

import concourse.bass as bass
import concourse.mybir as mybir
import concourse.tile as tile
from concourse import bacc
from concourse.bass_utils import run_bass_kernel_spmd
from concourse.masks import make_identity

F32 = mybir.dt.float32
BF16 = mybir.dt.bfloat16
AF = mybir.ActivationFunctionType
ALU = mybir.AluOpType

B, T, N, F_IN = 4, 8, 2000, 158
D, H, C = 128, 4, 32
HL = 2            # heads per core
NB = 16           # node blocks
TB = 125          # nodes per block
FA = F_IN + 1     # augmented features (ones col carries b_in)
KA = 128
KB = FA - KA      # 31
PW = HL * C + 2 * HL   # packed stage-1 cols: x0 (2 heads), e_src, e_dst
NEG_SLOPE = 0.2
# i-halves and PSUM 512-f32 bank chunking inside each half
IH = [(0, 1024), (1024, 2000)]
IHC = [[(0, 512), (512, 1024)], [(1024, 1536), (1536, 2000)]]

# tile-engine assignment: (jb % GPS_MOD) in GPS_SET -> gpsimd fused STT
GPS_MOD = 16
GPS_SET = frozenset()   # jb set routed to gpsimd (empty = all DVE)

_CACHE = {}


def ts(i, n):
    return slice(i * n, (i + 1) * n)


def _build_program(no_cc=False):
    nc = bacc.Bacc("TRN2", target_bir_lowering=False, debug=False, num_devices=8)

    xTa_d = nc.dram_tensor("xTa", [KA, N], F32, kind="ExternalInput")
    xTb_d = nc.dram_tensor("xTb", [KB, N], F32, kind="ExternalInput")
    gT_d = nc.dram_tensor("gT", [N, N], BF16, kind="ExternalInput")
    WiaA_d = nc.dram_tensor("WiaA", [KA, D], F32, kind="ExternalInput")
    WiaB_d = nc.dram_tensor("WiaB", [KB, D], F32, kind="ExternalInput")
    Wp0_d = nc.dram_tensor("Wp0", [D, PW], F32, kind="ExternalInput")
    Wp1_d = nc.dram_tensor("Wp1", [D, PW], F32, kind="ExternalInput")
    Wadb0_d = nc.dram_tensor("Wadb0", [D, HL * D], F32, kind="ExternalInput")
    Wadb1_d = nc.dram_tensor("Wadb1", [D, HL * D], F32, kind="ExternalInput")
    lngb_d = nc.dram_tensor("lngb", [128, D], F32, kind="ExternalInput")
    lnbb_d = nc.dram_tensor("lnbb", [128, D], F32, kind="ExternalInput")
    b0b_d = nc.dram_tensor("b0b", [128, D], F32, kind="ExternalInput")
    Woa_d = nc.dram_tensor("Woa", [C + 1, D], F32, kind="ExternalInput")
    out_d = nc.dram_tensor("out", [N, D], F32, kind="ExternalOutput")

    PAIRS = [[0, 1], [2, 3], [4, 5], [6, 7]]

    with tile.TileContext(nc) as tc, ExitStack() as ctx:
        persist = ctx.enter_context(tc.tile_pool(name="persist", bufs=1))
        work = ctx.enter_context(tc.tile_pool(name="work", bufs=3))
        tl_h = ctx.enter_context(tc.tile_pool(name="tl_h", bufs=3))
        tl_g = ctx.enter_context(tc.tile_pool(name="tl_g", bufs=5))
        ot_p = ctx.enter_context(tc.tile_pool(name="ot", bufs=1))
        psum = ctx.enter_context(tc.tile_pool(name="ps", bufs=2, space="PSUM"))
        psum_ag = ctx.enter_context(tc.tile_pool(name="psag", bufs=2, space="PSUM"))
        psum_m = ctx.enter_context(tc.tile_pool(name="psm", bufs=1, space="PSUM"))
        dram = ctx.enter_context(tc.tile_pool(name="dram", bufs=1, space="DRAM"))

        # ---- constants ----
        ident = persist.tile([128, 128], F32)
        make_identity(nc, ident)
        xTa = persist.tile([KA, N], F32)
        xTb = persist.tile([KB, N], F32)
        WiaA = persist.tile([KA, D], F32)
        WiaB = persist.tile([KB, D], F32)
        Wp = [persist.tile([D, PW], F32, name=f"Wp{l}", tag=f"Wp{l}")
              for l in range(2)]
        Wadb = [persist.tile([D, HL * D], F32, name=f"Wadb{l}", tag=f"Wadb{l}")
                for l in range(2)]
        lngb = persist.tile([128, D], F32)
        lnbb = persist.tile([128, D], F32)
        b0b = persist.tile([128, D], F32)
        Woa = persist.tile([C + 1, D], F32)
        for sb, dr in ((xTa, xTa_d), (xTb, xTb_d), (WiaA, WiaA_d), (WiaB, WiaB_d),
                       (Wp[0], Wp0_d), (Wp[1], Wp1_d), (Wadb[0], Wadb0_d),
                       (Wadb[1], Wadb1_d), (lngb, lngb_d), (lnbb, lnbb_d),
                       (b0b, b0b_d), (Woa, Woa_d)):
            nc.sync.dma_start(out=sb[:], in_=dr[:])

        gT = persist.tile([TB, NB, N], BF16)
        for jb in range(NB):
            nc.sync.dma_start(out=gT[:, jb, :], in_=gT_d[ts(jb, TB), :])

        # ---- persistent activations ----
        eps_t = persist.tile([TB, 1], F32)
        nc.vector.memset(eps_t[:], 1e-5)
        hN = persist.tile([TB, NB, D], F32)
        hT = persist.tile([D, N], F32)
        x0ext = persist.tile([TB, NB, HL, C + 1], BF16)
        es_N = persist.tile([TB, NB, HL], F32)
        esn = persist.tile([TB, NB, HL], F32)      # -e_src (compare scalar)
        edN = persist.tile([TB, NB, HL], F32)
        a1 = persist.tile([TB, NB, HL], F32)       # exp(e_dst)
        a2 = persist.tile([TB, NB, HL], F32)       # exp(0.2 e_dst)
        a2n = persist.tile([TB, NB, HL], F32)      # -exp(0.2 e_dst)
        esx1 = persist.tile([TB, NB, HL], BF16)    # exp(e_src)
        esx2 = persist.tile([TB, NB, HL], BF16)    # exp(0.2 e_src)
        xB12 = persist.tile([TB, NB, HL, 2 * (C + 1)], BF16)
        # m-stream stationary, padded so head h's rows start at partition
        # 64*h (engine APs need 32-aligned partition bases)
        xBg = persist.tile([TB, NB, 128], BF16)
        nc.vector.memset(xBg[:], 0.0)
        edb = persist.tile([128, HL, N], BF16)     # e_dst broadcast rows
        h0cat = persist.tile([TB, NB, HL * C], F32)
        hpacc = persist.tile([TB, NB, C], F32)
        h1aug = persist.tile([TB, NB, C + 1], F32)

        ag_in_d = dram.tile([N, HL * C], F32, tag="ag_in")
        ag_out_d = dram.tile([2, N, HL * C], F32, tag="ag_out")
        ar_in_d = dram.tile([N, C], F32, tag="ar_in")
        ar_out_d = dram.tile([N, C], F32, tag="ar_out")

        for jb in range(NB):
            nc.vector.memset(x0ext[:, jb, :, C:C + 1], 1.0)
            nc.vector.memset(h1aug[:, jb, C:C + 1], 1.0)

        # ---- stage 0: input projection + LN + ReLU -> hN, hT ----
        for nb in range(NB):
            ph = psum.tile([TB, D], F32, tag="ps")
            nc.tensor.matmul(ph[:], xTa[:, ts(nb, TB)], WiaA[:], start=True,
                             stop=False)
            nc.tensor.matmul(ph[:], xTb[:, ts(nb, TB)], WiaB[:], start=False,
                             stop=True)
            stats = work.tile([TB, 6], F32, tag="stats")
            nc.vector.bn_stats(out=stats[:], in_=ph[:])
            mv = work.tile([TB, 2], F32, tag="mv")
            nc.vector.bn_aggr(out=mv[:], in_=stats[:])
            sd = work.tile([TB, 1], F32, tag="sd")
            nc.scalar.activation(sd[:], mv[:, 1:2], AF.Sqrt,
                                 bias=eps_t[:, 0:1])
            rstd = work.tile([TB, 1], F32, tag="rstd")
            nc.vector.reciprocal(rstd[:], sd[:])
            hn = work.tile([TB, D], F32, tag="hn")
            nc.vector.tensor_scalar(out=hn[:], in0=ph[:], scalar1=mv[:, 0:1],
                                    scalar2=rstd[:, 0:1], op0=ALU.subtract,
                                    op1=ALU.mult)
            hg = work.tile([TB, D], F32, tag="hg")
            nc.vector.tensor_tensor(out=hg[:], in0=hn[:], in1=lngb[0:TB, :],
                                    op=ALU.mult)
            hb = work.tile([TB, D], F32, tag="hb")
            nc.vector.tensor_tensor(out=hb[:], in0=hg[:], in1=lnbb[0:TB, :],
                                    op=ALU.add)
            nc.vector.tensor_scalar(out=hN[:, nb, :], in0=hb[:], scalar1=0.0,
                                    scalar2=None, op0=ALU.max)
            pt = psum.tile([D, TB], F32, tag="ps")
            nc.tensor.transpose(pt[:], hN[:, nb, :], ident[0:TB, 0:TB])
            nc.scalar.copy(hT[:, ts(nb, TB)], pt[:])

        def stage1_block(l, nb):
            """x0 / e_src / e_dst for node block nb (from hT)."""
            px = psum.tile([TB, PW], F32, tag="ps")
            nc.tensor.matmul(px[:], hT[:, ts(nb, TB)], Wp[l][:],
                             start=True, stop=True)
            nc.scalar.copy(
                x0ext[:, nb, :, 0:C],
                px[:, 0:HL * C].rearrange("p (h c) -> p h c", h=HL))
            nc.scalar.copy(es_N[:, nb, :], px[:, HL * C:HL * C + HL])
            nc.scalar.copy(edN[:, nb, :], px[:, HL * C + HL:PW])

        def stage1_tail(l):
            """Per-layer smalls + edb + xB12/xBg, after all stage1 blocks."""
            esv = es_N[:].rearrange("p nb h -> p (nb h)")
            nc.vector.tensor_scalar(out=esn[:].rearrange("p nb h -> p (nb h)"),
                                    in0=esv, scalar1=-1.0, scalar2=None,
                                    op0=ALU.mult)
            edv = edN[:].rearrange("p nb h -> p (nb h)")
            nc.scalar.activation(a1[:].rearrange("p nb h -> p (nb h)"), edv,
                                 AF.Exp)
            nc.scalar.activation(a2[:].rearrange("p nb h -> p (nb h)"), edv,
                                 AF.Exp, scale=NEG_SLOPE)
            nc.vector.tensor_scalar(out=a2n[:].rearrange("p nb h -> p (nb h)"),
                                    in0=a2[:].rearrange("p nb h -> p (nb h)"),
                                    scalar1=-1.0, scalar2=None, op0=ALU.mult)
            nc.scalar.activation(esx1[:].rearrange("p nb h -> p (nb h)"), esv,
                                 AF.Exp)
            nc.scalar.activation(esx2[:].rearrange("p nb h -> p (nb h)"), esv,
                                 AF.Exp, scale=NEG_SLOPE)
            # edb: e_dst broadcast to all partitions via rank-1 matmul
            for h in range(HL):
                for q in range(4):
                    pe = psum.tile([128, 500], F32, tag="ps")
                    nc.tensor.matmul(pe[:], Wadb[l][:, ts(h, D)],
                                     hT[:, ts(q, 500)], start=True, stop=True)
                    nc.vector.tensor_copy(edb[:, h, ts(q, 500)], pe[:])
            # xB12 = [x0*e^{es} | x0*e^{.2es}] per head; xBg = both heads'
            # second halves, contiguous for the shared m-stream.
            for h in range(HL):
                for v, esx in ((0, esx1), (1, esx2)):
                    src = esx[:, :, h:h + 1]
                    bcast = bass.AP(tensor=src.tensor, offset=src.offset,
                                    ap=[src.ap[0], src.ap[1], [0, C + 1]])
                    nc.vector.tensor_tensor(
                        out=xB12[:].rearrange(
                            "p nb hh (v c) -> p nb hh v c", v=2)[:, :, h, v, :],
                        in0=x0ext[:, :, h, :], in1=bcast, op=ALU.mult)
                nc.vector.tensor_copy(xBg[:, :, 64 * h:64 * h + C + 1],
                                      xB12[:, :, h, C + 1:2 * (C + 1)])

        def make_g1(l, h, jb, ih, ia, ib_):
            """One G1 tile [TB, ia:ib_] for (head h, source block jb)."""
            g1 = tl_g.tile([TB, 1024], BF16, tag="g1")
            w = ib_ - ia
            ht = tl_h.tile([TB, 1024], BF16, tag="ht")
            nc.vector.tensor_scalar(out=ht[:, 0:w],
                                    in0=edb[0:TB, h, ia:ib_],
                                    scalar1=esn[:, jb, h:h + 1],
                                    scalar2=None, op0=ALU.is_ge)
            eng = nc.gpsimd if (jb % GPS_MOD) in GPS_SET else nc.vector
            eng.tensor_tensor(out=g1[:, 0:w], in0=ht[:, 0:w],
                              in1=gT[:, jb, ia:ib_], op=ALU.mult)
            return g1

        def gat_layer(l, oT12, oTg):
            """Both local heads; T-layout results into oT12[h] / shared oTg."""
            for ih, (ia, ib_) in enumerate(IH):
                # m-stream first: no DVE dependency, keeps PE warm
                mg = psum_m.tile([128, 1024], F32, tag="aggm")
                for jb in range(NB):
                    for (a, b_) in IHC[ih]:
                        nc.tensor.matmul(mg[:, a - ia:b_ - ia], xBg[:, jb, :],
                                         gT[:, jb, a:b_], start=(jb == 0),
                                         stop=(jb == NB - 1))
                for h in range(HL):
                    acc = psum_ag.tile([2 * (C + 1), 1024], F32, tag="agg")
                    for jb in range(NB):
                        g1 = make_g1(l, h, jb, ih, ia, ib_)
                        for (a, b_) in IHC[ih]:
                            nc.tensor.matmul(acc[:, a - ia:b_ - ia],
                                             xB12[:, jb, h, :],
                                             g1[:, a - ia:b_ - ia],
                                             start=(jb == 0),
                                             stop=(jb == NB - 1))
                    nc.scalar.copy(oT12[h][:, ia:ib_], acc[:, 0:ib_ - ia])
                nc.scalar.copy(oTg[:, ia:ib_], mg[:, 0:ib_ - ia])

        def tails(l, oT12, oTg):
            """Transpose back per block, combine branches, normalize."""
            for nb in range(NB):
                for h in range(HL):
                    ptc = psum.tile([TB, 3 * (C + 1)], F32, tag="ps")
                    nc.tensor.matmul(ptc[:, 0:2 * (C + 1)],
                                     oT12[h][:, ts(nb, TB)],
                                     ident[0:2 * (C + 1), 0:2 * (C + 1)],
                                     is_transpose=True, start=True, stop=False)
                    nc.tensor.matmul(ptc[:, 2 * (C + 1):],
                                     oTg[64 * h:64 * h + C + 1, ts(nb, TB)],
                                     ident[64 * h:64 * h + C + 1,
                                           64 * h:64 * h + C + 1],
                                     is_transpose=True, start=False, stop=True)
                    u1 = work.tile([TB, C + 1], F32, tag="u1")
                    nc.scalar.activation(u1[:], ptc[:, 0:C + 1], AF.Copy,
                                         scale=a1[:, nb, h:h + 1])
                    u2 = work.tile([TB, C + 1], F32, tag="u2")
                    nc.scalar.activation(u2[:], ptc[:, C + 1:2 * (C + 1)],
                                         AF.Copy, scale=a2n[:, nb, h:h + 1])
                    acc = work.tile([TB, C + 1], F32, tag="acc2")
                    nc.vector.tensor_tensor(out=acc[:], in0=u1[:], in1=u2[:],
                                            op=ALU.add)
                    hc = work.tile([TB, C + 1], F32, tag="hc")
                    nc.vector.scalar_tensor_tensor(
                        out=hc[:], in0=ptc[:, 2 * (C + 1):],
                        scalar=a2[:, nb, h:h + 1], in1=acc[:],
                        op0=ALU.mult, op1=ALU.add)
                    z = work.tile([TB, 1], F32, tag="z")
                    if l == 0:
                        nc.vector.tensor_copy(z[:], hc[:, C:C + 1])
                    else:
                        nc.vector.tensor_scalar(out=z[:], in0=hc[:, C:C + 1],
                                                scalar1=float(H), scalar2=None,
                                                op0=ALU.mult)
                    rz = work.tile([TB, 1], F32, tag="rz")
                    nc.vector.reciprocal(rz[:], z[:])
                    if l == 0:
                        nc.vector.tensor_scalar(
                            out=h0cat[:, nb, ts(h, C)], in0=hc[:, 0:C],
                            scalar1=rz[:, 0:1], scalar2=None, op0=ALU.mult)
                    elif h == 0:
                        nc.vector.tensor_scalar(
                            out=hpacc[:, nb, :], in0=hc[:, 0:C],
                            scalar1=rz[:, 0:1], scalar2=None, op0=ALU.mult)
                    else:
                        nc.vector.scalar_tensor_tensor(
                            out=hpacc[:, nb, :], in0=hc[:, 0:C],
                            scalar=rz[:, 0:1], in1=hpacc[:, nb, :],
                            op0=ALU.mult, op1=ALU.add)
                if l == 0:
                    nc.sync.dma_start(out=ag_in_d[ts(nb, TB), :],
                                      in_=h0cat[:, nb, :])
                else:
                    nc.sync.dma_start(out=ar_in_d[ts(nb, TB), :],
                                      in_=hpacc[:, nb, :])

        # ================= layer 0 =================
        for nb in range(NB):
            stage1_block(0, nb)
        stage1_tail(0)
        oT0 = [ot_p.tile([2 * (C + 1), N], F32, name=f"oT0_{h}", tag=f"oT{h}")
               for h in range(HL)]
        oTg0 = ot_p.tile([128, N], F32, name="oTg0", tag="oTg")
        gat_layer(0, oT0, oTg0)
        tails(0, oT0, oTg0)

        if no_cc:
            nc.gpsimd.dma_start(out=ag_out_d[0], in_=ag_in_d[:])
            nc.gpsimd.dma_start(out=ag_out_d[1], in_=ag_in_d[:])
        else:
            nc.gpsimd.collective_compute(
                "AllGather", ALU.bypass, replica_groups=PAIRS,
                ins=[ag_in_d[:].opt()], outs=[ag_out_d[:].opt()])

        # h_new = h + elu(h0 + bias0); rebuild hT, then layer-1 stage1
        for nb in range(NB):
            h0f = work.tile([TB, D], F32, tag="h0f")
            nc.sync.dma_start(out=h0f[:, 0:HL * C], in_=ag_out_d[0, ts(nb, TB), :])
            nc.sync.dma_start(out=h0f[:, HL * C:D], in_=ag_out_d[1, ts(nb, TB), :])
            h0b = work.tile([TB, D], F32, tag="h0b")
            nc.vector.tensor_tensor(out=h0b[:], in0=h0f[:], in1=b0b[0:TB, :],
                                    op=ALU.add)
            r2 = work.tile([TB, D], F32, tag="r2")
            nc.scalar.activation(r2[:], h0b[:], AF.Relu, scale=-1.0)
            ex = work.tile([TB, D], F32, tag="ex")
            nc.scalar.activation(ex[:], r2[:], AF.Exp, scale=-1.0)
            acc = work.tile([TB, D], F32, tag="acc")
            nc.vector.tensor_tensor(out=acc[:], in0=hN[:, nb, :], in1=h0b[:],
                                    op=ALU.add)
            nc.vector.tensor_tensor(out=acc[:], in0=acc[:], in1=r2[:], op=ALU.add)
            nc.vector.scalar_tensor_tensor(out=hN[:, nb, :], in0=ex[:],
                                           scalar=-1.0, in1=acc[:],
                                           op0=ALU.add, op1=ALU.add)
            pt1 = psum.tile([D, TB], F32, tag="ps")
            nc.tensor.transpose(pt1[:], hN[:, nb, :], ident[0:TB, 0:TB])
            nc.scalar.copy(hT[:, ts(nb, TB)], pt1[:])
            stage1_block(1, nb)

        # ================= layer 1 =================
        stage1_tail(1)
        oT1 = [ot_p.tile([2 * (C + 1), N], F32, name=f"oT1_{h}", tag=f"oT{h}")
               for h in range(HL)]
        oTg1 = ot_p.tile([128, N], F32, name="oTg1", tag="oTg")
        gat_layer(1, oT1, oTg1)
        tails(1, oT1, oTg1)

        if no_cc:
            nc.gpsimd.dma_start(out=ar_out_d[:], in_=ar_in_d[:])
        else:
            nc.gpsimd.collective_compute(
                "AllReduce", ALU.add, replica_groups=PAIRS,
                ins=[ar_in_d[:].opt()], outs=[ar_out_d[:].opt()])

        # final projection: out = h1 @ W_out + (bias1 @ W_out + b_out)
        for nb in range(NB):
            nc.sync.dma_start(out=h1aug[:, nb, 0:C], in_=ar_out_d[ts(nb, TB), :])
            ptc = psum.tile([C + 1, TB], F32, tag="ps")
            nc.tensor.transpose(ptc[:], h1aug[:, nb, :], ident[0:TB, 0:TB])
            lhs = work.tile([C + 1, TB], F32, tag="lhs")
            nc.vector.tensor_copy(lhs[:], ptc[:])
            po = psum.tile([TB, D], F32, tag="ps")
            nc.tensor.matmul(po[:], lhs[:], Woa[:], start=True, stop=True)
            ob = work.tile([TB, D], F32, tag="ob")
            nc.scalar.copy(ob[:], po[:])
            nc.sync.dma_start(out=out_d[ts(nb, TB), :], in_=ob[:])

    nc.compile()
    return nc


def _host_prep(inputs):
    """Build the 8 per-core input maps (pure numpy, not in HW time)."""
    f32 = np.float32
    x = np.asarray(inputs["x_alpha"], f32)[:, -1]            # [B, N, F_IN]
    sg = np.asarray(inputs["sector_graph"], f32)
    W_in = np.asarray(inputs["W_in"], f32)
    b_in = np.asarray(inputs["b_in"], f32)
    ln_g = np.asarray(inputs["ln_g"], f32)
    ln_b = np.asarray(inputs["ln_b"], f32)
    W0 = np.asarray(inputs["W0"], f32)
    as0 = np.asarray(inputs["as0"], f32)
    ad0 = np.asarray(inputs["ad0"], f32)
    W1 = np.asarray(inputs["W1"], f32)
    as1 = np.asarray(inputs["as1"], f32)
    ad1 = np.asarray(inputs["ad1"], f32)
    bias1 = np.asarray(inputs["bias1"], f32)
    W_out = np.asarray(inputs["W_out"], f32)
    b_out = np.asarray(inputs["b_out"], f32)
    bias0 = np.asarray(inputs["bias0"], f32)

    Wia = np.concatenate([W_in, b_in[None, :]], axis=0)       # [159, 128]
    lngb = np.ascontiguousarray(np.tile(ln_g[None, :], (128, 1)))
    lnbb = np.ascontiguousarray(np.tile(ln_b[None, :], (128, 1)))
    b0b = np.ascontiguousarray(np.tile(bias0[None, :], (128, 1)))
    Woa = np.concatenate([W_out, (bias1 @ W_out + b_out)[None, :]], axis=0)

    eye = np.eye(N, dtype=bool)
    in_maps = []
    for c in range(8):
        b = c // 2
        hp = c % 2
        heads = [2 * hp, 2 * hp + 1]
        xa = np.concatenate([x[b], np.ones((N, 1), f32)], axis=1)  # [N, 159]
        xT = np.ascontiguousarray(xa.T)                            # [159, N]
        mask = (sg[b] > 0) | eye
        gT = np.ascontiguousarray(mask.T).astype(ml_dtypes.bfloat16)

        def pack_p(W, as_, ad_):
            cols = [W[:, h * C:(h + 1) * C] for h in heads]
            cols += [(W[:, h * C:(h + 1) * C] @ as_[h])[:, None] for h in heads]
            cols += [(W[:, h * C:(h + 1) * C] @ ad_[h])[:, None] for h in heads]
            return np.ascontiguousarray(np.concatenate(cols, axis=1))

        def pack_db(W, ad_):
            cols = [np.repeat((W[:, h * C:(h + 1) * C] @ ad_[h])[:, None],
                              D, axis=1) for h in heads]
            return np.ascontiguousarray(np.concatenate(cols, axis=1))

        in_maps.append({
            "xTa": np.ascontiguousarray(xT[0:KA]),
            "xTb": np.ascontiguousarray(xT[KA:FA]),
            "gT": gT,
            "WiaA": np.ascontiguousarray(Wia[0:KA]),
            "WiaB": np.ascontiguousarray(Wia[KA:FA]),
            "Wp0": pack_p(W0, as0, ad0), "Wp1": pack_p(W1, as1, ad1),
            "Wadb0": pack_db(W0, ad0), "Wadb1": pack_db(W1, ad1),
            "lngb": lngb, "lnbb": lnbb, "b0b": b0b,
            "Woa": np.ascontiguousarray(Woa),
        })
    return in_maps


def kernel(**inputs):
    if "nc" not in _CACHE:
        _CACHE["nc"] = _build_program()
    nc = _CACHE["nc"]
    in_maps = _host_prep(inputs)
    res = run_bass_kernel_spmd(nc, in_maps, list(range(8)),
                               **_CACHE.get("run_kwargs", {}))
    _CACHE["last_results"] = res
    out = np.empty((B, N, D), np.float32)
    for b in range(B):
        out[b] = res.results[2 * b]["out"]
    return out
